# revision 1
# baseline (speedup 1.0000x reference)
"""GTLayer (graph transformer layer) distributed Bass kernel for 8 TRN2 cores.

Sharding: nodes (and their incoming edges) are partitioned across 8 cores by
node id (5000 dst nodes/core).  Host-side prep redistributes RAW input rows
per edge (the halo-exchange analog): for each core's dst-sorted, padded edge
list we build transposed per-edge arrays kT[e]=k[src_e], vT, qT(dst), efT.
The device does all model FLOPs: per-edge projections (Wk/Wv/Wq as stationary
matmul weights), edge-bias matmul, per-edge per-head dots (DVE), exp (ACT),
segment softmax-sum + weighted aggregation via one-hot matmuls into PSUM,
then Wo + residual + BN (global stats via AllReduce) + FFN + BN.
"""

import json
from contextlib import ExitStack
import numpy as np
import ml_dtypes

import concourse.bass as bass
import concourse.mybir as mybir
import concourse.tile as tile
from concourse.bass_utils import run_bass_kernel_spmd

bf16 = ml_dtypes.bfloat16

# problem constants (hardcoded per contract)
N, E, IN, H, D, ED = 40000, 640000, 128, 8, 16, 64
C = H * D            # 128
NCORE = 8
NSH = N // NCORE     # 5000 nodes per core
NG = 40              # node groups of <=128 per core (39*128+8)
SG_TILES = 20        # padded edge tiles per group (20*128 = 2560 slots)
SG = SG_TILES * 128
S = NG * SG          # slots per core
EPS = 1e-5

f32 = mybir.dt.float32
bft = mybir.dt.bfloat16


def _split_multiwaits_json(bir: bytes) -> bytes:
    """This walrus build allows only ONE sem wait per instruction; Tile emits
    multi-waits.  Split extras onto NoOps inserted before, same engine."""
    b = json.loads(bir)
    ctr = [0]
    changed = False
    for f in b.get("functions", []):
        for blk in f.get("blocks", []):
            insts = blk.get("instructions")
            if not insts:
                continue
            out = []
            for i in insts:
                si = i.get("sync_info")
                waits = (si or {}).get("on_wait") or []
                if len(waits) > 1:
                    changed = True
                    for w in waits[:-1]:
                        ctr[0] += 1
                        out.append({
                            "debug": i.get("debug", 0), "engine": i["engine"],
                            "ins": [], "name": f"I-wsplit-{ctr[0]}",
                            "opcode": "NoOp", "outs": [],
                            "text_hint": "wsplit",
                            "sync_info": {"on_update": [], "on_wait": [w]},
                        })
                    si["on_wait"] = [waits[-1]]
                out.append(i)
            blk["instructions"] = out
    return json.dumps(b).encode() if changed else bir


class _BassW(bass.Bass):
    def to_json_bytes(self) -> bytes:
        return _split_multiwaits_json(super().to_json_bytes())


def _build_program():
    nc = _BassW()
    dt_in = {
        "kT": (bft, [IN, S]), "vT": (bft, [IN, S]), "qeT": (bft, [IN, S]),
        "efT": (bft, [ED + 1, S]),
        "dstrel": (f32, [128, NG * SG_TILES]),
        "iota": (f32, [128, 128]),
        "qT": (f32, [IN, NSH]),
        "WkT": (bft, [IN, C]), "WvT": (bft, [IN, C]), "WqT": (bft, [IN, C]),
        "WeT": (bft, [ED + 1, H]),
        "WoT": (bft, [C, C]),
        "W1Ta": (bft, [C, C]), "W1Tb": (bft, [C, C]),
        "W2Ta": (bft, [C, C]), "W2Tb": (bft, [C, C]),
        "b1a": (f32, [128, 1]), "b1b": (f32, [128, 1]), "b2": (f32, [128, 1]),
        "g1": (f32, [128, 1]), "bt1": (f32, [128, 1]),
        "g2": (f32, [128, 1]), "bt2": (f32, [128, 1]),
    }
    dins = {k: nc.dram_tensor(k, sh, dt, kind="ExternalInput")
            for k, (dt, sh) in dt_in.items()}
    dout = nc.dram_tensor("out", [C, NSH], f32, kind="ExternalOutput")

    CH = 500  # phase-2 node chunk
    NCH = NSH // CH

    with tile.TileContext(nc) as tc:
        with (
            tc.tile_pool(name="const", bufs=1) as cpool,
            tc.tile_pool(name="wts", bufs=1) as wpool,
            tc.tile_pool(name="edge", bufs=2) as epool,
            tc.tile_pool(name="big", bufs=1) as bpool,
            tc.tile_pool(name="dram", bufs=1, space="DRAM") as dpool,
        ):
            # ---- constants / weights resident in SBUF ----
            iota_t = cpool.tile([128, 128], f32)
            nc.sync.dma_start(out=iota_t[:], in_=dins["iota"][:])
            w = {}
            for nm in ("WkT", "WvT", "WqT", "WoT", "W1Ta", "W1Tb", "W2Ta", "W2Tb"):
                w[nm] = wpool.tile([C, C], bft, name=nm, tag=nm)
                nc.sync.dma_start(out=w[nm][:], in_=dins[nm][:])
            we_t = wpool.tile([ED + 1, H], bft)
            nc.sync.dma_start(out=we_t[:], in_=dins["WeT"][:])
            vec = {}
            for nm in ("b1a", "b1b", "b2", "g1", "bt1", "g2", "bt2"):
                vec[nm] = wpool.tile([128, 1], f32, name=nm, tag=nm)
                nc.sync.dma_start(out=vec[nm][:], in_=dins[nm][:])
            qT_t = bpool.tile([IN, NSH], f32)
            nc.sync.dma_start(out=qT_t[:], in_=dins["qT"][:])

            # normalized aggregation output, channel-major, bf16
            aggT_sb = bpool.tile([C, NSH], bft)
            ident = cpool.tile([128, 128], bft)
            iota_col = cpool.tile([128, 1], mybir.dt.int32)
            nc.gpsimd.iota(iota_col[:], [[0, 1]], channel_multiplier=1)
            iota_col_f = cpool.tile([128, 1], f32)
            nc.vector.tensor_copy(iota_col_f[:], iota_col[:])
            nc.vector.tensor_tensor(
                out=ident[:], in0=iota_col_f[:].to_broadcast([128, 128]),
                in1=iota_t[:], op=mybir.AluOpType.is_equal)

            # ---- phase 1: per group ----
            ph1 = ExitStack()
            pspool = ph1.enter_context(tc.tile_pool(name="eps", bufs=1, space="PSUM"))
            aggpool = ph1.enter_context(tc.tile_pool(name="agg", bufs=1, space="PSUM"))
            for g in range(NG):
                n_lo = g * 128
                n_hi = min(NSH - n_lo, 128)
                agg_ps = aggpool.tile([128, C + H], f32)
                for t in range(SG_TILES):
                    e0 = g * SG + t * 128
                    kt = epool.tile([IN, 128], bft, tag="kt")
                    vt = epool.tile([IN, 128], bft, tag="vt")
                    qt = epool.tile([IN, 128], bft, tag="qt")
                    eft = epool.tile([ED + 1, 128], bft, tag="eft")
                    nc.sync.dma_start(out=kt[:], in_=dins["kT"][:, e0:e0 + 128])
                    nc.sync.dma_start(out=vt[:], in_=dins["vT"][:, e0:e0 + 128])
                    nc.sync.dma_start(out=qt[:], in_=dins["qeT"][:, e0:e0 + 128])
                    nc.sync.dma_start(out=eft[:], in_=dins["efT"][:, e0:e0 + 128])
                    dr = epool.tile([128, 1], f32, tag="dr")
                    nc.sync.dma_start(
                        out=dr[:], in_=dins["dstrel"][:, g * SG_TILES + t: g * SG_TILES + t + 1])

                    kp = pspool.tile([128, C], f32, tag="kp")
                    vp = pspool.tile([128, C], f32, tag="vp")
                    qp = pspool.tile([128, C], f32, tag="qp")
                    eb = pspool.tile([128, H], f32, tag="eb")
                    nc.tensor.matmul(kp[:], lhsT=kt[:], rhs=w["WkT"][:], start=True, stop=True)
                    nc.tensor.matmul(vp[:], lhsT=vt[:], rhs=w["WvT"][:], start=True, stop=True)
                    nc.tensor.matmul(qp[:], lhsT=qt[:], rhs=w["WqT"][:], start=True, stop=True)
                    nc.tensor.matmul(eb[:], lhsT=eft[:], rhs=we_t[:], start=True, stop=True)

                    # one-hot [e, n]
                    oh = epool.tile([128, 128], bft, tag="oh")
                    nc.vector.tensor_tensor(
                        out=oh[:], in0=dr[:].to_broadcast([128, 128]),
                        in1=iota_t[:], op=mybir.AluOpType.is_equal)

                    # scores
                    qps = epool.tile([128, C], f32, tag="qps")
                    nc.scalar.copy(qps[:], qp[:])
                    prod = epool.tile([128, C], f32, tag="prod")
                    nc.vector.tensor_tensor(out=prod[:], in0=kp[:], in1=qps[:],
                                            op=mybir.AluOpType.mult)
                    s0 = epool.tile([128, H], f32, tag="s0")
                    nc.vector.tensor_reduce(
                        out=s0[:], in_=prod[:].rearrange("p (h d) -> p h d", h=H),
                        axis=mybir.AxisListType.X, op=mybir.AluOpType.add)
                    sc = epool.tile([128, H], f32, tag="sc")
                    nc.vector.tensor_tensor(out=sc[:], in0=s0[:], in1=eb[:],
                                            op=mybir.AluOpType.add)
                    # rhs tile [Vw | ex]
                    rhs = epool.tile([128, C + H], bft, tag="rhs")
                    ex = rhs[:, C:C + H]
                    nc.scalar.activation(ex, sc[:], mybir.ActivationFunctionType.Exp)
                    nc.vector.tensor_tensor(
                        out=rhs[:, 0:C].rearrange("p (h d) -> p h d", h=H),
                        in0=vp[:].rearrange("p (h d) -> p h d", h=H),
                        in1=ex.to_broadcast([128, H, D]),
                        op=mybir.AluOpType.mult)
                    nc.tensor.matmul(agg_ps[:], lhsT=oh[:], rhs=rhs[:],
                                     start=(t == 0), stop=(t == SG_TILES - 1))

                # normalize by denominator and transpose to channel-major
                rec = epool.tile([128, H], f32, tag="rec")
                nc.vector.reciprocal(rec[:], agg_ps[:, C:C + H])
                aggn = epool.tile([128, C], bft, tag="aggn")
                nc.vector.tensor_tensor(
                    out=aggn[:].rearrange("p (h d) -> p h d", h=H),
                    in0=agg_ps[:, 0:C].rearrange("p (h d) -> p h d", h=H),
                    in1=rec[:].to_broadcast([128, H, D]),
                    op=mybir.AluOpType.mult)
                aggnT_ps = pspool.tile([128, 128], bft, tag="aggT")
                nc.tensor.transpose(aggnT_ps[:], aggn[:], ident[:])
                nc.vector.tensor_copy(aggT_sb[:, n_lo:n_lo + n_hi],
                                      aggnT_ps[:, 0:n_hi])

            ph1.close()
            # ---- phase 2: channel-major dense ----
            p2ctx = ExitStack()
            p2pool = p2ctx.enter_context(tc.tile_pool(name="ph2ps", bufs=1, space="PSUM"))
            rst = bpool.tile([C, NSH], f32)
            for ci in range(NCH):
                s0_ = ci * CH
                ps = p2pool.tile([128, CH], f32, tag="wo")
                nc.tensor.matmul(ps[:], lhsT=w["WoT"][:],
                                 rhs=aggT_sb[:, s0_:s0_ + CH], start=True, stop=True)
                nc.vector.tensor_tensor(out=rst[:, s0_:s0_ + CH], in0=ps[:],
                                        in1=qT_t[:, s0_:s0_ + CH],
                                        op=mybir.AluOpType.add)

            def bn_layer(x_sb, gv, btv, suffix):
                # global mean/var across all N nodes (AllReduce of sum/sumsq)
                st = bpool.tile([128, 2], f32, tag=f"st{suffix}")
                nc.vector.tensor_reduce(out=st[:, 0:1], in_=x_sb[:],
                                        axis=mybir.AxisListType.X,
                                        op=mybir.AluOpType.add)
                sq = bpool.tile([C, NSH], bft, tag="sqscratch")
                nc.scalar.activation(sq[:], x_sb[:],
                                     mybir.ActivationFunctionType.Square,
                                     accum_out=st[:, 1:2])
                bounce_in = dpool.tile([128, 2], f32, tag=f"bi{suffix}")
                bounce_out = dpool.tile([128, 2], f32, tag=f"bo{suffix}")
                nc.gpsimd.dma_start(out=bounce_in[:], in_=st[:])
                nc.gpsimd.collective_compute(
                    "AllReduce", mybir.AluOpType.add,
                    replica_groups=[list(range(NCORE))],
                    ins=[bounce_in.opt()], outs=[bounce_out.opt()])
                stg = bpool.tile([128, 2], f32, tag=f"stg{suffix}")
                nc.sync.dma_start(out=stg[:], in_=bounce_out[:])
                mean = bpool.tile([128, 1], f32, tag=f"mean{suffix}")
                nc.vector.tensor_scalar_mul(mean[:], stg[:, 0:1], 1.0 / N)
                msq = bpool.tile([128, 1], f32, tag=f"msq{suffix}")
                nc.scalar.activation(msq[:], mean[:],
                                     mybir.ActivationFunctionType.Square)
                var = bpool.tile([128, 1], f32, tag=f"var{suffix}")
                nc.vector.tensor_scalar_mul(var[:], stg[:, 1:2], 1.0 / N)
                nc.vector.tensor_tensor(out=var[:], in0=var[:], in1=msq[:],
                                        op=mybir.AluOpType.subtract)
                nc.vector.tensor_scalar_add(var[:], var[:], float(EPS))
                sd = bpool.tile([128, 1], f32, tag=f"sd{suffix}")
                nc.scalar.activation(sd[:], var[:],
                                     mybir.ActivationFunctionType.Sqrt)
                rsd = bpool.tile([128, 1], f32, tag=f"rsd{suffix}")
                nc.vector.reciprocal(rsd[:], sd[:])
                scale = bpool.tile([128, 1], f32, tag=f"scale{suffix}")
                nc.vector.tensor_tensor(out=scale[:], in0=rsd[:], in1=gv[:],
                                        op=mybir.AluOpType.mult)
                nmean = bpool.tile([128, 1], f32, tag=f"nm{suffix}")
                nc.vector.tensor_tensor(out=nmean[:], in0=mean[:], in1=scale[:],
                                        op=mybir.AluOpType.mult)
                shift = bpool.tile([128, 1], f32, tag=f"shift{suffix}")
                nc.vector.tensor_tensor(out=shift[:], in0=btv[:], in1=nmean[:],
                                        op=mybir.AluOpType.subtract)
                return scale, shift

            sc1, sh1 = bn_layer(rst, vec["g1"], vec["bt1"], "1")
            xbn = bpool.tile([C, NSH], f32)
            nc.scalar.activation(xbn[:], rst[:],
                                 mybir.ActivationFunctionType.Identity,
                                 bias=sh1[:], scale=sc1[:])
            xbn_bf = bpool.tile([C, NSH], bft)
            nc.vector.tensor_copy(xbn_bf[:], xbn[:])

            y = bpool.tile([C, NSH], f32)
            for ci in range(NCH):
                s0_ = ci * CH
                rhs2 = xbn_bf[:, s0_:s0_ + CH]
                h1a = p2pool.tile([128, CH], f32, tag="h1a")
                h1b = p2pool.tile([128, CH], f32, tag="h1b")
                nc.tensor.matmul(h1a[:], lhsT=w["W1Ta"][:], rhs=rhs2, start=True, stop=True)
                nc.tensor.matmul(h1b[:], lhsT=w["W1Tb"][:], rhs=rhs2, start=True, stop=True)
                r1a = epool.tile([128, CH], bft, tag="r1a")
                r1b = epool.tile([128, CH], bft, tag="r1b")
                nc.scalar.activation(r1a[:], h1a[:],
                                     mybir.ActivationFunctionType.Relu,
                                     bias=vec["b1a"][:])
                nc.scalar.activation(r1b[:], h1b[:],
                                     mybir.ActivationFunctionType.Relu,
                                     bias=vec["b1b"][:])
                h2 = p2pool.tile([128, CH], f32, tag="h2")
                nc.tensor.matmul(h2[:], lhsT=w["W2Ta"][:], rhs=r1a[:], start=True, stop=False)
                nc.tensor.matmul(h2[:], lhsT=w["W2Tb"][:], rhs=r1b[:], start=False, stop=True)
                # y = h2 + b2 + xbn
                yt = epool.tile([128, CH], f32, tag="yt")
                nc.scalar.activation(yt[:], h2[:],
                                     mybir.ActivationFunctionType.Identity,
                                     bias=vec["b2"][:])
                nc.vector.tensor_tensor(out=y[:, s0_:s0_ + CH], in0=yt[:],
                                        in1=xbn[:, s0_:s0_ + CH],
                                        op=mybir.AluOpType.add)

            sc2, sh2 = bn_layer(y, vec["g2"], vec["bt2"], "2")
            yout = bpool.tile([C, NSH], f32)
            nc.scalar.activation(yout[:], y[:],
                                 mybir.ActivationFunctionType.Identity,
                                 bias=sh2[:], scale=sc2[:])
            nc.sync.dma_start(out=dout[:], in_=yout[:])
            p2ctx.close()
    return nc


def _host_prep(q, k, v, edge_feat, src, dst, Wq, Wk, Wv, We, be, Wo,
               W1, b1, W2, b2, g1, bt1, g2, bt2):
    order = np.argsort(dst, kind="stable")
    src_s = src[order]
    dst_s = dst[order]
    ef_s = edge_feat[order]

    in_maps = []
    for m in range(NCORE):
        lo, hi = m * NSH, (m + 1) * NSH
        sel = (dst_s >= lo) & (dst_s < hi)
        srcm, dstm, efm = src_s[sel], dst_s[sel] - lo, ef_s[sel]
        # slot layout: per group g, SG slots
        kT = np.zeros((IN, S), dtype=bf16)
        vT = np.zeros((IN, S), dtype=bf16)
        qeT = np.zeros((IN, S), dtype=bf16)
        efT = np.zeros((ED + 1, S), dtype=bf16)
        dstrel = np.full((128, NG * SG_TILES), -1.0, dtype=np.float32)
        grp = dstm // 128
        for g in range(NG):
            gs = np.nonzero(grp == g)[0]
            ne = len(gs)
            assert ne <= SG, f"group {g} core {m} has {ne} edges > SG={SG}"
            base = g * SG
            kT[:, base:base + ne] = k[srcm[gs]].T
            vT[:, base:base + ne] = v[srcm[gs]].T
            qeT[:, base:base + ne] = q[dstm[gs] + lo].T
            efT[:ED, base:base + ne] = efm[gs].T
            efT[ED, base:base + ne] = 1.0
            rel = (dstm[gs] - g * 128).astype(np.float32)
            sl = np.arange(ne)
            dstrel[sl % 128, g * SG_TILES + sl // 128] = rel
        iota = np.broadcast_to(np.arange(128, dtype=np.float32), (128, 128)).copy()
        im = {
            "kT": kT, "vT": vT, "qeT": qeT, "efT": efT,
            "dstrel": dstrel, "iota": iota,
            "qT": q[lo:hi].T.astype(np.float32).copy(),
            "WkT": Wk.T.astype(bf16).copy(),
            "WvT": Wv.T.astype(bf16).copy(),
            "WqT": (Wq / np.sqrt(np.float32(D))).T.astype(bf16).copy(),
            "WeT": np.concatenate([We.T, be[None, :]], 0).astype(bf16).copy(),
            "WoT": Wo.T.astype(bf16).copy(),
            "W1Ta": W1[:C].T.astype(bf16).copy(),
            "W1Tb": W1[C:].T.astype(bf16).copy(),
            "W2Ta": W2.T[:C].astype(bf16).copy(),
            "W2Tb": W2.T[C:].astype(bf16).copy(),
            "b1a": b1[:C, None].astype(np.float32).copy(),
            "b1b": b1[C:, None].astype(np.float32).copy(),
            "b2": b2[:, None].astype(np.float32).copy(),
            "g1": g1[:, None].astype(np.float32).copy(),
            "bt1": bt1[:, None].astype(np.float32).copy(),
            "g2": g2[:, None].astype(np.float32).copy(),
            "bt2": bt2[:, None].astype(np.float32).copy(),
        }
        in_maps.append(im)
    return in_maps


RUN_KW = {}
LAST = {}


def kernel(**inputs):
    inputs = {k: np.asarray(v) for k, v in inputs.items()}
    in_maps = _host_prep(**inputs)
    nc = _build_program()
    res = run_bass_kernel_spmd(nc, in_maps, core_ids=list(range(NCORE)),
                               **RUN_KW)
    LAST["res"] = res
    out = np.concatenate([r["out"].T for r in res.results], axis=0)
    return out.astype(np.float32)



# revision 4
# speedup vs baseline: 1.0725x; 1.0725x over previous
"""GTLayer (graph transformer layer) distributed Bass kernel for 8 TRN2 cores.

Sharding: nodes (and their incoming edges) are partitioned across 8 cores by
node id (5000 dst nodes/core).  Host-side prep redistributes RAW input rows
per edge (the halo-exchange analog): for each core's dst-sorted, padded edge
list we build transposed per-edge arrays kT[e]=k[src_e], vT, qT(dst), efT.
The device does all model FLOPs: per-edge projections (Wk/Wv/Wq as stationary
matmul weights), edge-bias matmul, per-edge per-head dots (DVE), exp (ACT),
segment softmax-sum + weighted aggregation via one-hot matmuls into PSUM,
then Wo + residual + BN (global stats via AllReduce) + FFN + BN.
"""

import json
from contextlib import ExitStack
import numpy as np
import ml_dtypes

import concourse.bass as bass
import concourse.mybir as mybir
import concourse.tile as tile
from concourse.bass_utils import run_bass_kernel_spmd

bf16 = ml_dtypes.bfloat16

# problem constants (hardcoded per contract)
N, E, IN, H, D, ED = 40000, 640000, 128, 8, 16, 64
C = H * D            # 128
NCORE = 8
NSH = N // NCORE     # 5000 nodes per core
NG = 40              # node groups of <=128 per core (39*128+8)
SG_TILES = 20        # padded edge tiles per group (20*128 = 2560 slots)
SG = SG_TILES * 128
S = NG * SG          # slots per core
EPS = 1e-5

f32 = mybir.dt.float32
bft = mybir.dt.bfloat16


def _split_multiwaits_json(bir: bytes) -> bytes:
    """This walrus build allows only ONE sem wait per instruction; Tile emits
    multi-waits.  Split extras onto NoOps inserted before, same engine."""
    b = json.loads(bir)
    ctr = [0]
    changed = False
    for f in b.get("functions", []):
        for blk in f.get("blocks", []):
            insts = blk.get("instructions")
            if not insts:
                continue
            out = []
            for i in insts:
                si = i.get("sync_info")
                waits = (si or {}).get("on_wait") or []
                if len(waits) > 1:
                    changed = True
                    for w in waits[:-1]:
                        ctr[0] += 1
                        out.append({
                            "debug": i.get("debug", 0), "engine": i["engine"],
                            "ins": [], "name": f"I-wsplit-{ctr[0]}",
                            "opcode": "NoOp", "outs": [],
                            "text_hint": "wsplit",
                            "sync_info": {"on_update": [], "on_wait": [w]},
                        })
                    si["on_wait"] = [waits[-1]]
                out.append(i)
            blk["instructions"] = out
    return json.dumps(b).encode() if changed else bir


class _BassW(bass.Bass):
    def to_json_bytes(self) -> bytes:
        return _split_multiwaits_json(super().to_json_bytes())


TPB = 3 * 128          # blob cols per tile: kt | vt | qt
GPB = SG_TILES * TPB   # blob cols per group


def _build_program():
    nc = _BassW()
    dt_in = {
        "blob": (bft, [IN, NG * GPB]),
        "efT": (bft, [ED + 1, S]),
        "dstrel": (f32, [128, NG * SG_TILES]),
        "iota": (f32, [128, 128]),
        "qT": (f32, [IN, NSH]),
        "WkT": (bft, [IN, C]), "WvT": (bft, [IN, C]), "WqT": (bft, [IN, C]),
        "WeT": (bft, [ED + 1, H]),
        "WoT": (bft, [C, C]),
        "W1Ta": (bft, [C, C]), "W1Tb": (bft, [C, C]),
        "W2Ta": (bft, [C, C]), "W2Tb": (bft, [C, C]),
        "b1a": (f32, [128, 1]), "b1b": (f32, [128, 1]), "b2": (f32, [128, 1]),
        "g1": (f32, [128, 1]), "bt1": (f32, [128, 1]),
        "g2": (f32, [128, 1]), "bt2": (f32, [128, 1]),
    }
    dins = {k: nc.dram_tensor(k, sh, dt, kind="ExternalInput")
            for k, (dt, sh) in dt_in.items()}
    dout = nc.dram_tensor("out", [C, NSH], f32, kind="ExternalOutput")

    CH = 500  # phase-2 node chunk
    NCH = NSH // CH

    with tile.TileContext(nc) as tc:
        with (
            tc.tile_pool(name="const", bufs=1) as cpool,
            tc.tile_pool(name="wts", bufs=1) as wpool,
            tc.tile_pool(name="edge", bufs=2) as epool,
            tc.tile_pool(name="big", bufs=1) as bpool,
            tc.tile_pool(name="dram", bufs=1, space="DRAM") as dpool,
        ):
            # ---- constants / weights resident in SBUF ----
            iota_t = cpool.tile([128, 128], f32)
            nc.sync.dma_start(out=iota_t[:], in_=dins["iota"][:])
            w = {}
            for nm in ("WkT", "WvT", "WqT", "WoT", "W1Ta", "W1Tb", "W2Ta", "W2Tb"):
                w[nm] = wpool.tile([C, C], bft, name=nm, tag=nm)
                nc.sync.dma_start(out=w[nm][:], in_=dins[nm][:])
            we_t = wpool.tile([ED + 1, H], bft)
            nc.sync.dma_start(out=we_t[:], in_=dins["WeT"][:])
            vec = {}
            for nm in ("b1a", "b1b", "b2", "g1", "bt1", "g2", "bt2"):
                vec[nm] = wpool.tile([128, 1], f32, name=nm, tag=nm)
                nc.sync.dma_start(out=vec[nm][:], in_=dins[nm][:])
            qT_t = bpool.tile([IN, NSH], f32)
            nc.sync.dma_start(out=qT_t[:], in_=dins["qT"][:])

            # normalized aggregation output, channel-major, bf16
            aggT_sb = bpool.tile([C, NSH], bft)
            ident = cpool.tile([128, 128], bft)
            iota_col = cpool.tile([128, 1], mybir.dt.int32)
            nc.gpsimd.iota(iota_col[:], [[0, 1]], channel_multiplier=1)
            iota_col_f = cpool.tile([128, 1], f32)
            nc.vector.tensor_copy(iota_col_f[:], iota_col[:])
            nc.vector.tensor_tensor(
                out=ident[:], in0=iota_col_f[:].to_broadcast([128, 128]),
                in1=iota_t[:], op=mybir.AluOpType.is_equal)

            # ---- phase 1: per group ----
            ph1 = ExitStack()
            pspool = ph1.enter_context(tc.tile_pool(name="eps", bufs=1, space="PSUM"))
            aggpool = ph1.enter_context(tc.tile_pool(name="agg", bufs=1, space="PSUM"))
            for g in range(NG):
                n_lo = g * 128
                n_hi = min(NSH - n_lo, 128)
                agg_ps = aggpool.tile([128, C + H], f32)
                # one DMA each for the group's packed k|v|q blob, edge
                # features, and relative-dst columns
                gblob = epool.tile([IN, GPB], bft, tag="gblob")
                geft = epool.tile([ED + 1, SG], bft, tag="geft")
                gdr = epool.tile([128, SG_TILES], f32, tag="gdr")
                nc.sync.dma_start(out=gblob[:], in_=dins["blob"][:, g * GPB:(g + 1) * GPB])
                nc.sync.dma_start(out=geft[:], in_=dins["efT"][:, g * SG:(g + 1) * SG])
                nc.sync.dma_start(
                    out=gdr[:], in_=dins["dstrel"][:, g * SG_TILES:(g + 1) * SG_TILES])
                for t in range(SG_TILES):
                    kt = gblob[:, t * TPB:t * TPB + 128]
                    vt = gblob[:, t * TPB + 128:t * TPB + 256]
                    qt = gblob[:, t * TPB + 256:t * TPB + 384]
                    eft = geft[:, t * 128:(t + 1) * 128]
                    dr = gdr[:, t:t + 1]

                    kp = pspool.tile([128, C], f32, tag="kp")
                    vp = pspool.tile([128, C], f32, tag="vp")
                    qp = pspool.tile([128, C], f32, tag="qp")
                    eb = pspool.tile([128, H], f32, tag="eb")
                    nc.tensor.matmul(kp[:], lhsT=kt, rhs=w["WkT"][:], start=True, stop=True)
                    nc.tensor.matmul(vp[:], lhsT=vt, rhs=w["WvT"][:], start=True, stop=True)
                    nc.tensor.matmul(qp[:], lhsT=qt, rhs=w["WqT"][:], start=True, stop=True)
                    nc.tensor.matmul(eb[:], lhsT=eft, rhs=we_t[:], start=True, stop=True)

                    # one-hot [e, n]
                    oh = epool.tile([128, 128], bft, tag="oh")
                    nc.vector.tensor_tensor(
                        out=oh[:], in0=dr.to_broadcast([128, 128]),
                        in1=iota_t[:], op=mybir.AluOpType.is_equal)

                    # scores
                    qps = epool.tile([128, C], f32, tag="qps")
                    nc.scalar.copy(qps[:], qp[:])
                    prod = epool.tile([128, C], f32, tag="prod")
                    nc.vector.tensor_tensor(out=prod[:], in0=kp[:], in1=qps[:],
                                            op=mybir.AluOpType.mult)
                    s0 = epool.tile([128, H], f32, tag="s0")
                    nc.vector.tensor_reduce(
                        out=s0[:], in_=prod[:].rearrange("p (h d) -> p h d", h=H),
                        axis=mybir.AxisListType.X, op=mybir.AluOpType.add)
                    sc = epool.tile([128, H], f32, tag="sc")
                    nc.vector.tensor_tensor(out=sc[:], in0=s0[:], in1=eb[:],
                                            op=mybir.AluOpType.add)
                    # rhs tile [Vw | ex]
                    rhs = epool.tile([128, C + H], bft, tag="rhs")
                    ex = rhs[:, C:C + H]
                    nc.scalar.activation(ex, sc[:], mybir.ActivationFunctionType.Exp)
                    nc.vector.tensor_tensor(
                        out=rhs[:, 0:C].rearrange("p (h d) -> p h d", h=H),
                        in0=vp[:].rearrange("p (h d) -> p h d", h=H),
                        in1=ex.to_broadcast([128, H, D]),
                        op=mybir.AluOpType.mult)
                    nc.tensor.matmul(agg_ps[:], lhsT=oh[:], rhs=rhs[:],
                                     start=(t == 0), stop=(t == SG_TILES - 1))

                # normalize by denominator and transpose to channel-major
                rec = epool.tile([128, H], f32, tag="rec")
                nc.vector.reciprocal(rec[:], agg_ps[:, C:C + H])
                aggn = epool.tile([128, C], bft, tag="aggn")
                nc.vector.tensor_tensor(
                    out=aggn[:].rearrange("p (h d) -> p h d", h=H),
                    in0=agg_ps[:, 0:C].rearrange("p (h d) -> p h d", h=H),
                    in1=rec[:].to_broadcast([128, H, D]),
                    op=mybir.AluOpType.mult)
                aggnT_ps = pspool.tile([128, 128], bft, tag="aggT")
                nc.tensor.transpose(aggnT_ps[:], aggn[:], ident[:])
                nc.vector.tensor_copy(aggT_sb[:, n_lo:n_lo + n_hi],
                                      aggnT_ps[:, 0:n_hi])

            ph1.close()
            # ---- phase 2: channel-major dense ----
            p2ctx = ExitStack()
            p2pool = p2ctx.enter_context(tc.tile_pool(name="ph2ps", bufs=1, space="PSUM"))
            rst = bpool.tile([C, NSH], f32)
            for ci in range(NCH):
                s0_ = ci * CH
                ps = p2pool.tile([128, CH], f32, tag="wo")
                nc.tensor.matmul(ps[:], lhsT=w["WoT"][:],
                                 rhs=aggT_sb[:, s0_:s0_ + CH], start=True, stop=True)
                nc.vector.tensor_tensor(out=rst[:, s0_:s0_ + CH], in0=ps[:],
                                        in1=qT_t[:, s0_:s0_ + CH],
                                        op=mybir.AluOpType.add)

            def bn_layer(x_sb, gv, btv, suffix):
                # global mean/var across all N nodes (AllReduce of sum/sumsq)
                st = bpool.tile([128, 2], f32, tag=f"st{suffix}")
                nc.vector.tensor_reduce(out=st[:, 0:1], in_=x_sb[:],
                                        axis=mybir.AxisListType.X,
                                        op=mybir.AluOpType.add)
                sq = bpool.tile([C, NSH], bft, tag="sqscratch")
                nc.scalar.activation(sq[:], x_sb[:],
                                     mybir.ActivationFunctionType.Square,
                                     accum_out=st[:, 1:2])
                bounce_in = dpool.tile([128, 2], f32, tag=f"bi{suffix}")
                bounce_out = dpool.tile([128, 2], f32, tag=f"bo{suffix}")
                nc.gpsimd.dma_start(out=bounce_in[:], in_=st[:])
                nc.gpsimd.collective_compute(
                    "AllReduce", mybir.AluOpType.add,
                    replica_groups=[list(range(NCORE))],
                    ins=[bounce_in.opt()], outs=[bounce_out.opt()])
                stg = bpool.tile([128, 2], f32, tag=f"stg{suffix}")
                nc.sync.dma_start(out=stg[:], in_=bounce_out[:])
                mean = bpool.tile([128, 1], f32, tag=f"mean{suffix}")
                nc.vector.tensor_scalar_mul(mean[:], stg[:, 0:1], 1.0 / N)
                msq = bpool.tile([128, 1], f32, tag=f"msq{suffix}")
                nc.scalar.activation(msq[:], mean[:],
                                     mybir.ActivationFunctionType.Square)
                var = bpool.tile([128, 1], f32, tag=f"var{suffix}")
                nc.vector.tensor_scalar_mul(var[:], stg[:, 1:2], 1.0 / N)
                nc.vector.tensor_tensor(out=var[:], in0=var[:], in1=msq[:],
                                        op=mybir.AluOpType.subtract)
                nc.vector.tensor_scalar_add(var[:], var[:], float(EPS))
                sd = bpool.tile([128, 1], f32, tag=f"sd{suffix}")
                nc.scalar.activation(sd[:], var[:],
                                     mybir.ActivationFunctionType.Sqrt)
                rsd = bpool.tile([128, 1], f32, tag=f"rsd{suffix}")
                nc.vector.reciprocal(rsd[:], sd[:])
                scale = bpool.tile([128, 1], f32, tag=f"scale{suffix}")
                nc.vector.tensor_tensor(out=scale[:], in0=rsd[:], in1=gv[:],
                                        op=mybir.AluOpType.mult)
                nmean = bpool.tile([128, 1], f32, tag=f"nm{suffix}")
                nc.vector.tensor_tensor(out=nmean[:], in0=mean[:], in1=scale[:],
                                        op=mybir.AluOpType.mult)
                shift = bpool.tile([128, 1], f32, tag=f"shift{suffix}")
                nc.vector.tensor_tensor(out=shift[:], in0=btv[:], in1=nmean[:],
                                        op=mybir.AluOpType.subtract)
                return scale, shift

            sc1, sh1 = bn_layer(rst, vec["g1"], vec["bt1"], "1")
            xbn = bpool.tile([C, NSH], f32)
            nc.scalar.activation(xbn[:], rst[:],
                                 mybir.ActivationFunctionType.Identity,
                                 bias=sh1[:], scale=sc1[:])
            xbn_bf = bpool.tile([C, NSH], bft)
            nc.vector.tensor_copy(xbn_bf[:], xbn[:])

            y = bpool.tile([C, NSH], f32)
            for ci in range(NCH):
                s0_ = ci * CH
                rhs2 = xbn_bf[:, s0_:s0_ + CH]
                h1a = p2pool.tile([128, CH], f32, tag="h1a")
                h1b = p2pool.tile([128, CH], f32, tag="h1b")
                nc.tensor.matmul(h1a[:], lhsT=w["W1Ta"][:], rhs=rhs2, start=True, stop=True)
                nc.tensor.matmul(h1b[:], lhsT=w["W1Tb"][:], rhs=rhs2, start=True, stop=True)
                r1a = epool.tile([128, CH], bft, tag="r1a")
                r1b = epool.tile([128, CH], bft, tag="r1b")
                nc.scalar.activation(r1a[:], h1a[:],
                                     mybir.ActivationFunctionType.Relu,
                                     bias=vec["b1a"][:])
                nc.scalar.activation(r1b[:], h1b[:],
                                     mybir.ActivationFunctionType.Relu,
                                     bias=vec["b1b"][:])
                h2 = p2pool.tile([128, CH], f32, tag="h2")
                nc.tensor.matmul(h2[:], lhsT=w["W2Ta"][:], rhs=r1a[:], start=True, stop=False)
                nc.tensor.matmul(h2[:], lhsT=w["W2Tb"][:], rhs=r1b[:], start=False, stop=True)
                # y = h2 + b2 + xbn
                yt = epool.tile([128, CH], f32, tag="yt")
                nc.scalar.activation(yt[:], h2[:],
                                     mybir.ActivationFunctionType.Identity,
                                     bias=vec["b2"][:])
                nc.vector.tensor_tensor(out=y[:, s0_:s0_ + CH], in0=yt[:],
                                        in1=xbn[:, s0_:s0_ + CH],
                                        op=mybir.AluOpType.add)

            sc2, sh2 = bn_layer(y, vec["g2"], vec["bt2"], "2")
            yout = bpool.tile([C, NSH], f32)
            nc.scalar.activation(yout[:], y[:],
                                 mybir.ActivationFunctionType.Identity,
                                 bias=sh2[:], scale=sc2[:])
            nc.sync.dma_start(out=dout[:], in_=yout[:])
            p2ctx.close()
    return nc


def _host_prep(q, k, v, edge_feat, src, dst, Wq, Wk, Wv, We, be, Wo,
               W1, b1, W2, b2, g1, bt1, g2, bt2):
    order = np.argsort(dst, kind="stable")
    src_s = src[order]
    dst_s = dst[order]
    ef_s = edge_feat[order]

    in_maps = []
    for m in range(NCORE):
        lo, hi = m * NSH, (m + 1) * NSH
        sel = (dst_s >= lo) & (dst_s < hi)
        srcm, dstm, efm = src_s[sel], dst_s[sel] - lo, ef_s[sel]
        # slot layout: per group g, SG slots; blob interleaves kt|vt|qt
        # per 128-edge tile so one DMA per group moves all three
        blob = np.zeros((IN, NG * GPB), dtype=bf16)
        blob4 = blob.reshape(IN, NG, SG_TILES, 3, 128)
        efT = np.zeros((ED + 1, S), dtype=bf16)
        dstrel = np.full((128, NG * SG_TILES), -1.0, dtype=np.float32)
        grp = dstm // 128
        for g in range(NG):
            gs = np.nonzero(grp == g)[0]
            ne = len(gs)
            assert ne <= SG, f"group {g} core {m} has {ne} edges > SG={SG}"
            base = g * SG
            kTg = np.zeros((IN, SG), dtype=bf16)
            vTg = np.zeros((IN, SG), dtype=bf16)
            qTg = np.zeros((IN, SG), dtype=bf16)
            kTg[:, :ne] = k[srcm[gs]].T
            vTg[:, :ne] = v[srcm[gs]].T
            qTg[:, :ne] = q[dstm[gs] + lo].T
            blob4[:, g, :, 0, :] = kTg.reshape(IN, SG_TILES, 128)
            blob4[:, g, :, 1, :] = vTg.reshape(IN, SG_TILES, 128)
            blob4[:, g, :, 2, :] = qTg.reshape(IN, SG_TILES, 128)
            efT[:ED, base:base + ne] = efm[gs].T
            efT[ED, base:base + ne] = 1.0
            rel = (dstm[gs] - g * 128).astype(np.float32)
            sl = np.arange(ne)
            dstrel[sl % 128, g * SG_TILES + sl // 128] = rel
        iota = np.broadcast_to(np.arange(128, dtype=np.float32), (128, 128)).copy()
        im = {
            "blob": blob, "efT": efT,
            "dstrel": dstrel, "iota": iota,
            "qT": q[lo:hi].T.astype(np.float32).copy(),
            "WkT": Wk.T.astype(bf16).copy(),
            "WvT": Wv.T.astype(bf16).copy(),
            "WqT": (Wq / np.sqrt(np.float32(D))).T.astype(bf16).copy(),
            "WeT": np.concatenate([We.T, be[None, :]], 0).astype(bf16).copy(),
            "WoT": Wo.T.astype(bf16).copy(),
            "W1Ta": W1[:C].T.astype(bf16).copy(),
            "W1Tb": W1[C:].T.astype(bf16).copy(),
            "W2Ta": W2.T[:C].astype(bf16).copy(),
            "W2Tb": W2.T[C:].astype(bf16).copy(),
            "b1a": b1[:C, None].astype(np.float32).copy(),
            "b1b": b1[C:, None].astype(np.float32).copy(),
            "b2": b2[:, None].astype(np.float32).copy(),
            "g1": g1[:, None].astype(np.float32).copy(),
            "bt1": bt1[:, None].astype(np.float32).copy(),
            "g2": g2[:, None].astype(np.float32).copy(),
            "bt2": bt2[:, None].astype(np.float32).copy(),
        }
        in_maps.append(im)
    return in_maps


RUN_KW = {}
LAST = {}


def kernel(**inputs):
    inputs = {k: np.asarray(v) for k, v in inputs.items()}
    in_maps = _host_prep(**inputs)
    nc = _build_program()
    res = run_bass_kernel_spmd(nc, in_maps, core_ids=list(range(NCORE)),
                               **RUN_KW)
    LAST["res"] = res
    out = np.concatenate([r["out"].T for r in res.results], axis=0)
    return out.astype(np.float32)

